# revision 46
# baseline (speedup 1.0000x reference)
"""Trainium2 Bass kernel for nn_Attention_39436389712179 (sparse_attention).

Sharding: 8-way tensor parallel over heads (2 heads / core).
 - wq/wk/wv/wky/wvy column-sharded by head; wo row-sharded; gate with heads.
 - q/k LayerNorm couples all 2048 channels -> per-core partial (sum, sumsq)
   stats + one tiny AllReduce ([6, R] f32).

Wire-minimal IO (the platform ships every external input/output buffer
per call at ~15 GB/s with per-array overheads and a 3x penalty for
odd-sized arrays, while on-device DMA/compute/collectives are fast):
 - everything ships as ONE padded f16 blob per core (region offsets from
   _blob_layout; f32 tensors ride as bit-viewed f16 pairs).
 - x / y ship SHARDED (1/8 per core) as 12-bit planes (hi byte + packed
   low nibbles, ~bf16 precision) and are AllGathered on device, then
   decoded to f16 in SBUF by DVE integer ops at first use.
 - weights ship as 12-bit planes too, decoded once into SBUF.
 - the per-core [R, D] f16 output partials are ReduceScattered on device
   in two halves (the first overlaps the second half's matmuls); each
   core ships only its f16 [R/8, D] shard, reassembled by unshard_out.

Layout: feature-major ("T") activations [channels, rows]; attention matmuls
run in float32r (f32 data, bf16-rate on PE), projections in fp16.
RoPE channels are deinterleaved (evens then odds per head) by permuting the
q/k/ky weight columns host-side, so the rotation becomes partition-block ops.
Softmax runs max-free (scores are O(1) after LN) with the row-sum computed by
an ones-vector matmul, and the 1/sum normalization applied to the PV output.
"""
import math
import sys
from contextlib import ExitStack

import numpy as np

sys.path.insert(0, "/opt/trn_rl_repo")

from concourse import bacc
import concourse.tile as tile
import concourse.mybir as mybir
from concourse.tile_rust import add_dep_helper

F32 = mybir.dt.float32
F32R = mybir.dt.float32r
F16 = mybir.dt.float16
U8 = mybir.dt.uint8
U16 = mybir.dt.uint16
AF = mybir.ActivationFunctionType
ALU = mybir.AluOpType

# Full problem config
B_F, S_F, D_F, H_F, HD_F, LY_F, DY_F = 2, 2048, 2048, 16, 128, 512, 2048
NCORES = 8
HPC = H_F // NCORES          # heads per core = 2
C = HPC * HD_F               # channels per core = 256
HHD = H_F * HD_F             # LayerNorm width = 2048
EPS_QK = 1e-5
EPS_KY = 1e-6

TRACE = False                # test.py sets True to collect exec time
_BUILD_CACHE = {}


def _cfg_full():
    return dict(B=B_F, S=S_F, D=D_F, LY=LY_F, DY=DY_F)


def _blob_layout(cfg):
    """Offsets (in f16 elements) of each packed input region; every region
    is padded to a 512-element boundary (odd-sized transfers are ~3x slower
    through the shipping layer). f32 tensors occupy 2 elements per value.
    Regions named *_hi / *_lo are 12-bit planes: _hi holds each f16 value's
    high byte (1 byte/value), _lo the low nibbles (1 byte per value PAIR);
    sizes there are in VALUES and converted to f16 slots (2 values/slot)."""
    B, S, D, LY, DY = cfg["B"], cfg["S"], cfg["D"], cfg["LY"], cfg["DY"]
    R, RY = B * S, B * LY
    DCH, DYCH = D // NCORES, DY // NCORES
    sizes = [
        ("xg_hi", DCH * R // 2),
        ("xg_lo", DCH * R // 4),
        ("yg_hi", DYCH * RY // 2),
        ("yg_lo", DYCH * RY // 4),
        ("csg", (128 // NCORES) * S),
        ("wq_hi", D * C // 2), ("wq_lo", D * C // 4),
        ("wk_hi", D * C // 2), ("wk_lo", D * C // 4),
        ("wv_hi", D * C // 2), ("wv_lo", D * C // 4),
        ("wky_hi", DY * C // 2), ("wky_lo", DY * C // 4),
        ("wvy_hi", DY * C // 2), ("wvy_lo", DY * C // 4),
        ("wo_hi", C * D // 2), ("wo_lo", C * D // 4),
        ("gam", 65 * C * 2),
        ("nbcol", 128 * 3 * HPC * 2),
        ("gate", 65 * 1 * 2),
    ]
    lay = {}
    off = 0
    for name, n in sizes:
        lay[name] = (off, n)
        off += (n + 511) // 512 * 512
    ntot = (off + 4095) // 4096 * 4096
    return lay, ntot


def _pack12(arr):
    """f16 array -> (hi u8 plane, lo-nibble-pair u8 plane), rounding to
    12 bits (round-half-up on the dropped 4 mantissa bits)."""
    bits = np.ascontiguousarray(arr.astype(np.float16)).view(np.uint16)
    bits = ((bits.astype(np.uint32) + 8) & 0xFFF0).astype(np.uint16)
    flat = bits.ravel()
    hi = (flat >> 8).astype(np.uint8)
    lonib = ((flat >> 4) & 0xF).astype(np.uint8)
    lo = (lonib[0::2] | (lonib[1::2] << 4)).astype(np.uint8)
    return hi, lo


def build(cfg, bench_mode=False):
    B, S, D, LY, DY = cfg["B"], cfg["S"], cfg["D"], cfg["LY"], cfg["DY"]
    R = B * S
    RY = B * LY
    NDB = D // 128            # d-blocks for x projections
    NYB = DY // 128
    NST = R // 512            # 512-col tiles over all rows
    NYST = RY // 512
    NJ = S // 512             # q chunks per batch
    NT = S // 128             # self-attn key tiles per batch
    NTY = LY // 128           # cross-attn key tiles per batch
    DCH = D // NCORES         # x-shard rows per core
    DYCH = DY // NCORES
    RSH = R // NCORES         # output rows per core
    assert R % 512 == 0 and RY % 512 == 0 and S % 512 == 0
    assert LY % 128 == 0 and LY <= 512
    assert D % NCORES == 0 and DY % NCORES == 0 and R % NCORES == 0
    assert RSH % 128 == 0 and 128 % NCORES == 0

    nc = bacc.Bacc("TRN2", target_bir_lowering=False,
                   num_devices=1 if bench_mode else NCORES)

    # ALL inputs ship as ONE padded f16 blob per core (the transfer layer
    # charges ~1ms per array and 3x for odd-sized ones; f32 tensors are
    # bit-viewed as f16 pairs and bitcast back on device)
    lay, NTOT = _blob_layout(cfg)
    blob = nc.dram_tensor("blob", [1, NTOT], F16, kind="ExternalInput")

    def bview(name, rows, dtype=F16):
        off, n = lay[name]
        ap = blob[0:1, off:off + n].rearrange("p (a b) -> (p a) b", a=rows)
        if dtype is not F16:
            ap = ap.bitcast(dtype)
        return ap

    def bview8(name, rows, chunk=None, nchunks=1):
        """u8 view of a plane region (or its chunk-th 1/nchunks slice),
        shaped [rows, *]."""
        off, n = lay[name]
        if chunk is not None:
            n = n // nchunks
            off = off + chunk * n
        return (blob[0:1, off:off + n].bitcast(U8)
                .rearrange("p (a b) -> (p a) b", a=rows))

    # output ships 12-bit packed: hi-byte plane then low-nibble-pair plane
    NOUT_HI = RSH * D // 2            # f16 slots (1 byte/value)
    NOUT = NOUT_HI + RSH * D // 4     # + 1 byte per value pair
    out_d = nc.dram_tensor("out_sl", [1, NOUT], F16, kind="ExternalOutput")

    _sp = "Local" if bench_mode else "Shared"
    stats_shA = nc.dram_tensor("stats_shA", [6, R // 2], F32, addr_space=_sp)
    stats_shB = nc.dram_tensor("stats_shB", [6, R // 2], F32, addr_space=_sp)
    xA_sh = nc.dram_tensor("xA_sh", [D, R], U8, addr_space=_sp)
    xB_sh = nc.dram_tensor("xB_sh", [D, R // 2], U8, addr_space=_sp)
    yA_sh = nc.dram_tensor("yA_sh", [DY, RY], U8, addr_space=_sp)
    yB_sh = nc.dram_tensor("yB_sh", [DY, RY // 2], U8, addr_space=_sp)
    cs_sh = nc.dram_tensor("cs_sh", [128, S], F16, addr_space=_sp)
    rsout_d = nc.dram_tensor("rsout", [RSH, D], F16, addr_space="Local")

    with tile.TileContext(nc) as tc, ExitStack() as _top:
        if True:
            cp = _top.enter_context(tc.tile_pool(name="consts", bufs=1))
            dp = _top.enter_context(tc.tile_pool(name="dram", bufs=1, space="DRAM"))

            # ---- gather the sharded inputs (device-side; wire-free) ----
            # collectives may not read IO tensors: stage shards into Local
            # DRAM pool tiles (dependency-tracked) first
            xA_loc = dp.tile([DCH, R], U8, tag="xA_loc")
            xB_loc = dp.tile([DCH, R // 2], U8, tag="xB_loc")
            yA_loc = dp.tile([DYCH, RY], U8, tag="yA_loc")
            yB_loc = dp.tile([DYCH, RY // 2], U8, tag="yB_loc")
            csg_loc = dp.tile([128 // NCORES, S], F16, tag="csg_loc")

            def _gather(src_ap, loc, dst, nrows):
                nc.sync.dma_start(loc[:, :], src_ap)
                if bench_mode:
                    nc.sync.dma_start(dst[0:nrows, :], loc[:, :])
                else:
                    nc.gpsimd.collective_compute(
                        "AllGather", ALU.bypass,
                        replica_groups=[list(range(NCORES))],
                        ins=[loc[:, :].opt()], outs=[dst[:, :].opt()])

            # y/cs first: phase 1 starts with the y projections, so the
            # small gathers must not queue behind the big x planes
            _gather(bview8("yg_hi", DYCH), yA_loc, yA_sh, DYCH)
            _gather(bview8("yg_lo", DYCH), yB_loc, yB_sh, DYCH)
            _gather(bview("csg", 128 // NCORES), csg_loc, cs_sh,
                    128 // NCORES)
            _gather(bview8("xg_hi", DCH), xA_loc, xA_sh, DCH)
            _gather(bview8("xg_lo", DCH), xB_loc, xB_sh, DCH)

            # ---- 12-bit decode helper: dst[p,n] f16 <- hi[p,n], lo[p,n/2] --
            # grp>1: the plane loads use 3-D (grouped-block) access patterns
            def dec12(pool, dst_ap, hi_ap, lo_ap, p, n, bufs=2, grp=1):
                a8 = pool.tile([p, n], U8, tag="dc_a8", bufs=bufs)
                a8_dst = (a8[:] if grp == 1 else
                          a8[:].rearrange("p (g s) -> p g s", g=grp))
                nc.sync.dma_start(a8_dst, hi_ap)
                b8 = pool.tile([p, n // 2], U8, tag="dc_b8", bufs=bufs)
                b8_dst = (b8[:] if grp == 1 else
                          b8[:].rearrange("p (g s) -> p g s", g=grp))
                nc.sync.dma_start(b8_dst, lo_ap)
                a16 = pool.tile([p, n], U16, tag="dc_a16", bufs=bufs)
                nc.vector.tensor_copy(a16[:], a8[:])
                du = dst_ap.bitcast(U16)
                nc.vector.tensor_scalar(out=du, in0=a16[:], scalar1=8,
                                        scalar2=None,
                                        op0=ALU.logical_shift_left)
                b16 = pool.tile([p, n // 2], U16, tag="dc_b16", bufs=bufs)
                nc.vector.tensor_copy(b16[:], b8[:])
                lo0 = pool.tile([p, n // 2], U16, tag="dc_lo0", bufs=bufs)
                nc.vector.tensor_scalar(out=lo0[:], in0=b16[:], scalar1=0xF,
                                        scalar2=4, op0=ALU.bitwise_and,
                                        op1=ALU.logical_shift_left)
                lo1 = pool.tile([p, n // 2], U16, tag="dc_lo1", bufs=bufs)
                nc.vector.tensor_scalar(out=lo1[:], in0=b16[:], scalar1=0xF0,
                                        scalar2=None, op0=ALU.bitwise_and)
                duv = du.rearrange("p (n two) -> p n two", two=2)
                nc.vector.tensor_tensor(
                    out=duv[:, :, 0:1], in0=duv[:, :, 0:1],
                    in1=lo0[:].rearrange("p (n o) -> p n o", o=1),
                    op=ALU.bitwise_or)
                nc.vector.tensor_tensor(
                    out=duv[:, :, 1:2], in0=duv[:, :, 1:2],
                    in1=lo1[:].rearrange("p (n o) -> p n o", o=1),
                    op=ALU.bitwise_or)



            # ---- constants ----
            # cos/sin: cs_sh rows [0:64]=cos, [64:128]=sin; duplicate each
            # 64-row half into a [128, S] f32 tile (deinterleaved RoPE).
            _csw = ExitStack()
            csp = _csw.enter_context(tc.tile_pool(name="csstage", bufs=1))
            cs16 = csp.tile([128, S], F16, tag="cs16")
            nc.sync.dma_start(cs16[0:64, :], cs_sh[0:64, :])
            nc.sync.dma_start(cs16[64:128, :], cs_sh[64:128, :])
            cos2_t = cp.tile([128, S], F32, tag="cos2")
            nc.vector.tensor_copy(cos2_t[0:64, :], cs16[0:64, :])
            nc.vector.tensor_copy(cos2_t[64:128, :], cs16[0:64, :])
            sin2_t = cp.tile([128, S], F32, tag="sin2")
            nc.vector.tensor_copy(sin2_t[0:64, :], cs16[64:128, :])
            nc.vector.tensor_copy(sin2_t[64:128, :], cs16[64:128, :])
            gam_t = cp.tile([65, C], F32R, tag="gam")
            nc.sync.dma_start(gam_t[:], bview("gam", 65, F32R))
            nbcol_t = cp.tile([128, 3 * HPC], F32, tag="nbcol")
            nc.sync.dma_start(nbcol_t[:], bview("nbcol", 128, F32))

            ones_col32 = cp.tile([1, 128], F32, tag="onc32")
            nc.vector.memset(ones_col32[:], 1.0)
            ones_col = cp.tile([1, 128], F32R, tag="onc")
            nc.vector.tensor_copy(ones_col[:], ones_col32[:])
            ones_row32 = cp.tile([128, 1], F32, tag="onr32")
            nc.vector.memset(ones_row32[:], 1.0)
            ones_row = cp.tile([128, 1], F32R, tag="onr")
            nc.vector.tensor_copy(ones_row[:], ones_row32[:])
            eps_t = cp.tile([65, 1], F32, tag="eps")
            nc.vector.memset(eps_t[:], EPS_QK)
            nc.vector.memset(eps_t[64:65, :], EPS_KY)
            gate_t = cp.tile([65, 1], F32, tag="gate")
            nc.sync.dma_start(gate_t[:], bview("gate", 65, F32))
            g_t = cp.tile([65, 1], F32, tag="gtanh")
            nc.scalar.activation(g_t[:], gate_t[:], AF.Tanh)
            # prewarm ACT function tables during the DMA-bound start so the
            # first real Sqrt/Exp/Square/Identity doesn't pay the table-set
            # load (~2.7us each) on the critical path
            g_rows = []
            for _hl in range(HPC):
                g_row = cp.tile([1, 128], F32R, tag=f"grow{_hl}",
                                name=f"grow{_hl}")
                nc.vector.tensor_scalar(
                    out=g_row[:], in0=ones_col32[:],
                    scalar1=g_t[32 * _hl:32 * _hl + 1, 0:1],
                    scalar2=None, op0=ALU.mult)
                g_rows.append(g_row)
            warm = cp.tile([1, 4], F32, tag="actwarm")
            nc.vector.memset(warm[:], 1.0)
            for _fn in (AF.Square, AF.Sqrt, AF.Identity, AF.Exp):
                nc.scalar.activation(warm[:], warm[:], _fn)
            # LN coefficient tiles (filled in phase 1S)
            rs_t = cp.tile([65, R], F32R, tag="rs")
            mrs_t = cp.tile([65, R], F32R, tag="mrs")
            # stats work tiles: pre-memset early, freed after phase 1S
            _sw = ExitStack()
            smw = _sw.enter_context(tc.tile_pool(name="statw", bufs=1))
            sums_t = smw.tile([65, R], F32, tag="sums")
            nc.vector.memset(sums_t[:], 1.0)
            sq_t = smw.tile([65, R], F32, tag="sqs")
            nc.vector.memset(sq_t[:], 1.0)

            # ---- DRAM scratch ----
            q_raw_dr = dp.tile([C, R], F32, tag="q_raw")
            k_raw_dr = dp.tile([C, R], F32, tag="k_raw")
            yk_raw_dr = dp.tile([C, RY], F32, tag="yk_raw")
            v_dr = dp.tile([R, C], F32, tag="v")
            yv_dr = dp.tile([RY, C], F32, tag="yv")
            o_dr = dp.tile([C, R], F32, tag="o")
            part_dr = dp.tile([R, D], F16, tag="part")
            stats_drA = dp.tile([6, R // 2], F32, tag="statsA")
            stats_drB = dp.tile([6, R // 2], F32, tag="statsB")

            # =================== PHASE 1: projections + stats ===============
            with ExitStack() as _s1:
                wp = _s1.enter_context(tc.tile_pool(name="wx", bufs=1))
                xp = _s1.enter_context(tc.tile_pool(name="xt", bufs=3))
                rawp = _s1.enter_context(tc.tile_pool(name="raw", bufs=6))
                sqp = _s1.enter_context(tc.tile_pool(name="sq", bufs=2))
                smallp = _s1.enter_context(tc.tile_pool(name="small", bufs=4))
                pps = _s1.enter_context(tc.tile_pool(name="pps", bufs=6, space="PSUM"))
                stps = _s1.enter_context(tc.tile_pool(name="stps", bufs=2, space="PSUM"))
                wq_sb = wp.tile([128, NDB * C], F16, tag="wq")
                wk_sb = wp.tile([128, NDB * C], F16, tag="wk")
                wv_sb = wp.tile([128, NDB * C], F16, tag="wv")
                wky_sb = wp.tile([128, NYB * C], F16, tag="wky")
                wvy_sb = wp.tile([128, NYB * C], F16, tag="wvy")

                wdp = _s1.enter_context(tc.tile_pool(name="wdec", bufs=3))

                def load_w_chunk(w_sb, w_name, dblk, ndb_):
                    dec12(wdp, w_sb[:, dblk * C:(dblk + 1) * C],
                          bview8(w_name + "_hi", 128, dblk, ndb_),
                          bview8(w_name + "_lo", 128, dblk, ndb_),
                          128, C, bufs=3)

                def proj_tile(src_planes, w_list, v_spec, st, ndb,
                              wload=None):
                    """One 512-col tile of projections.

                    w_list: [(w_sb, psum_pair, spill_dr, stat_rows)] for the
                    weight-stationary q/k-style outputs (T-layout + stats).
                    v_spec: (wv_sb, spill_dr) -> natural-layout output via
                    activation-stationary matmuls (no transpose needed).
                    """
                    col = st * 512
                    srcA, srcB = src_planes
                    vw_sb, v_spill = v_spec
                    vps_pair = [pps.tile([128, 512], F32, tag="proj",
                                         name="vprojp") for _ in range(2)]
                    xt_grp = None
                    GRP = 2 if ndb % 2 == 0 else 1
                    for dblk in range(ndb):
                        if wload is not None:
                            wload(dblk)
                        if dblk % GRP == 0:
                            xt_grp = xp.tile([128, GRP * 512], F16, tag="xt")
                            g0 = dblk
                            dec12(
                                xp, xt_grp[:],
                                srcA[dblk * 128:(dblk + GRP) * 128,
                                     col:col + 512]
                                .rearrange("(n p) s -> p n s", p=128),
                                srcB[dblk * 128:(dblk + GRP) * 128,
                                     col // 2:col // 2 + 256]
                                .rearrange("(n p) s -> p n s", p=128),
                                128, GRP * 512, bufs=3, grp=GRP)
                        xt = xt_grp[:, (dblk - g0) * 512:(dblk - g0 + 1) * 512]
                        for w_sb, pst, _sp, _st in w_list:
                            for cb in range(2):
                                nc.tensor.matmul(
                                    pst[cb][:],
                                    w_sb[:, dblk * C + cb * 128:
                                         dblk * C + cb * 128 + 128],
                                    xt,
                                    start=(dblk == 0), stop=(dblk == ndb - 1))
                        for sub in range(4):
                            # two seq-subtiles share one PSUM bank (= one
                            # 2KB zero region): only sub%2==0 sets start;
                            # the partner's first write consumes the same
                            # pending-zero. Order the pair explicitly.
                            mm = nc.tensor.matmul(
                                vps_pair[sub // 2][:, (sub % 2) * 256:
                                                   (sub % 2) * 256 + 256],
                                xt[:, sub * 128:(sub + 1) * 128],
                                vw_sb[:, dblk * C:dblk * C + 256],
                                start=(dblk == 0 and sub % 2 == 0),
                                stop=(dblk == ndb - 1),
                                skip_group_check=True)
                            if dblk == 0:
                                if sub % 2 == 0:
                                    first_vmm = mm
                                else:
                                    add_dep_helper(
                                        mm.ins, first_vmm.ins,
                                        reason="psum zero-region pair order")
                    # v: PSUM holds [seq128, ch256] pairs; copy + one 3-D DMA
                    for half in range(2):
                        vsb = rawp.tile([128, 512], F32, tag="raw")
                        nc.scalar.copy(vsb[:], vps_pair[half][:])
                        nc.scalar.dma_start(
                            v_spill[col + half * 256:col + half * 256 + 256, :]
                            .rearrange("(s p) c -> p s c", p=128),
                            vsb[:].rearrange("p (s c) -> p s c", s=2))
                    for w_sb, pst, spill_dr, stat_rows in w_list:
                        st_sum = stps.tile([1, 512], F32, tag="stat")
                        st_sq = stps.tile([1, 512], F32, tag="stat")
                        for cb in range(2):
                            raw = rawp.tile([128, 512], F32R, tag="raw")
                            nc.vector.tensor_copy(raw[:], pst[cb][:])
                            nc.scalar.dma_start(
                                spill_dr[cb * 128:(cb + 1) * 128,
                                         col:col + 512],
                                raw[:].bitcast(F32))
                            nc.tensor.matmul(st_sum[:], ones_row[:], raw[:],
                                             start=(cb == 0), stop=(cb == 1))
                            sq = sqp.tile([128, 512], F32R, tag="sq")
                            nc.scalar.activation(sq[:], raw[:].bitcast(F32),
                                                 AF.Square)
                            nc.tensor.matmul(st_sq[:], ones_row[:], sq[:],
                                             start=(cb == 0), stop=(cb == 1))
                        r0, r1 = stat_rows
                        sdr, scol = ((stats_drA, col) if col < R // 2
                                     else (stats_drB, col - R // 2))
                        s0 = smallp.tile([1, 512], F32, tag="small")
                        nc.vector.tensor_copy(s0[:], st_sum[:])
                        nc.gpsimd.dma_start(sdr[r0:r0 + 1, scol:scol + 512],
                                            s0[:])
                        s1 = smallp.tile([1, 512], F32, tag="small")
                        nc.vector.tensor_copy(s1[:], st_sq[:])
                        nc.gpsimd.dma_start(sdr[r1:r1 + 1, scol:scol + 512],
                                            s1[:])

                # zero-fill unused y-stat columns first (independent)
                z = smallp.tile([1, 512], F32, tag="small")
                nc.vector.memset(z[:], 0.0)
                for col in range(RY, R, 512):
                    sdr, scol = ((stats_drA, col) if col < R // 2
                                 else (stats_drB, col - R // 2))
                    nc.gpsimd.dma_start(sdr[4:5, scol:scol + 512], z[:])
                    nc.gpsimd.dma_start(sdr[5:6, scol:scol + 512], z[:])

                def _ar(buf, shared):
                    if bench_mode:
                        nc.sync.dma_start(shared[:, :], buf[:])
                    else:
                        nc.gpsimd.collective_compute(
                            "AllReduce", ALU.add,
                            replica_groups=[list(range(NCORES))],
                            ins=[buf[:].opt()], outs=[shared[:, :].opt()])

                # y projections first: their stats live in the first half
                for st in range(NYST):
                    ykps = [pps.tile([128, 512], F32, tag="proj", name="projp") for _ in range(2)]
                    proj_tile((yA_sh, yB_sh),
                              [(wky_sb, ykps, yk_raw_dr, (4, 5))],
                              (wvy_sb, yv_dr), st, NYB,
                              wload=(lambda dblk: (load_w_chunk(wky_sb, "wky", dblk, NYB),
                                                   load_w_chunk(wvy_sb, "wvy", dblk, NYB))
                                     if st == 0 else None))
                for st in range(NST):
                    qps = [pps.tile([128, 512], F32, tag="proj", name="projp") for _ in range(2)]
                    kps = [pps.tile([128, 512], F32, tag="proj", name="projp") for _ in range(2)]
                    proj_tile((xA_sh, xB_sh),
                              [(wq_sb, qps, q_raw_dr, (0, 1)),
                               (wk_sb, kps, k_raw_dr, (2, 3))],
                              (wv_sb, v_dr), st, NDB,
                              wload=(lambda dblk: (load_w_chunk(wq_sb, "wq", dblk, NDB),
                                                   load_w_chunk(wk_sb, "wk", dblk, NDB),
                                                   load_w_chunk(wv_sb, "wv", dblk, NDB))
                                     if st == 0 else None))
                    if st == NST // 2 - 1:
                        _ar(stats_drA, stats_shA)
                _ar(stats_drB, stats_shB)

            # =================== PHASE 1S: LN statistics ====================
            with tc.tile_pool(name="statm", bufs=1) as smp:
                for half, sh in enumerate((stats_shA, stats_shB)):
                    hc = half * (R // 2)
                    hs_ = slice(hc, hc + R // 2)
                    for i, row in enumerate((0, 2, 4)):
                        nc.sync.dma_start(sums_t[32 * i:32 * i + 1, hs_],
                                          sh[row:row + 1, :])
                    for i, row in enumerate((1, 3, 5)):
                        nc.sync.dma_start(sq_t[32 * i:32 * i + 1, hs_],
                                          sh[row:row + 1, :])
                    mu = smp.tile([65, R // 2], F32, tag="mu")
                    nc.scalar.mul(mu[:], sums_t[:, hs_], 1.0 / HHD)
                    mu2 = smp.tile([65, R // 2], F32, tag="mu2")
                    nc.vector.tensor_mul(mu2[:], mu[:], mu[:])
                    var = smp.tile([65, R // 2], F32, tag="var")
                    nc.vector.scalar_tensor_tensor(
                        var[:], sq_t[:, hs_], 1.0 / HHD, mu2[:],
                        op0=ALU.mult, op1=ALU.subtract)
                    sig = smp.tile([65, R // 2], F32, tag="sig")
                    nc.scalar.activation(sig[:], var[:], AF.Sqrt,
                                         bias=eps_t[:, 0:1], scale=1.0)
                    with nc.allow_low_precision(
                            reason="f32r holds full f32 bits"):
                        nc.vector.reciprocal(rs_t[:, hs_], sig[:])
                    nc.vector.tensor_mul(mrs_t[:, hs_], mu[:],
                                         rs_t[:, hs_].bitcast(F32))
            _sw.close()
            _csw.close()

            # =================== PHASE 2: attention =========================
            with ExitStack() as _s3:
                bigp = _s3.enter_context(tc.tile_pool(name="big", bufs=2))
                ykfp = _s3.enter_context(tc.tile_pool(name="ykf", bufs=2))
                lnp = _s3.enter_context(tc.tile_pool(name="lnraw", bufs=2))
                tmpp = _s3.enter_context(tc.tile_pool(name="lntmp", bufs=4))
                vp = _s3.enter_context(tc.tile_pool(name="vtl", bufs=2))
                yvp = _s3.enter_context(tc.tile_pool(name="yvtl", bufs=2))
                ptp = _s3.enter_context(tc.tile_pool(name="ptile", bufs=4))
                obp = _s3.enter_context(tc.tile_pool(name="osb", bufs=4))
                rcp = _s3.enter_context(tc.tile_pool(name="rcs", bufs=3))
                sp_ = _s3.enter_context(tc.tile_pool(name="sps", bufs=3, space="PSUM"))
                coefp = sp_
                OpsP = _s3.enter_context(tc.tile_pool(name="Ops", bufs=2, space="PSUM"))
                O2psP = _s3.enter_context(tc.tile_pool(name="O2ps", bufs=1, space="PSUM"))
                sumP = _s3.enter_context(tc.tile_pool(name="sums", bufs=1, space="PSUM"))
                sum2P = _s3.enter_context(tc.tile_pool(name="sums2", bufs=1, space="PSUM"))
                def ln_chunk(dst, dst_col, rawt, base, hl, col0, j, do_rope,
                             jl=None):
                    col = col0 + j * 512
                    hs = hl * 128
                    nb_i = (base // 32) * HPC + hl
                    jl = j if jl is None else jl
                    raw = rawt[:, jl * 512:(jl + 1) * 512]
                    a_ps = O2psP.tile([128, 512], F32, tag="O2")
                    nc.tensor.matmul(a_ps[:], gam_t[base:base + 1, hs:hs + 128],
                                     rs_t[base:base + 1, col:col + 512],
                                     start=True, stop=True)
                    b_ps = O2psP.tile([128, 512], F32, tag="O2")
                    nc.tensor.matmul(b_ps[:], gam_t[base:base + 1, hs:hs + 128],
                                     mrs_t[base:base + 1, col:col + 512],
                                     start=True, stop=True)
                    a_sb = tmpp.tile([128, 512], F32, tag="coefsb", bufs=4)
                    nc.scalar.copy(a_sb[:], a_ps[:])
                    b_sb = tmpp.tile([128, 512], F32, tag="coefsb", bufs=4)
                    # beta folded in: b_sb = (gamma*mu*rsig) + (-beta)
                    nc.scalar.activation(b_sb[:], b_ps[:], AF.Identity,
                                         bias=nbcol_t[:, nb_i:nb_i + 1],
                                         scale=1.0)
                    t1 = tmpp.tile([128, 512], F32, tag="lntmp")
                    nc.vector.tensor_mul(t1[:], raw, a_sb[:])
                    if not do_rope:
                        nc.vector.tensor_sub(dst[:, dst_col:dst_col + 512],
                                             t1[:], b_sb[:])
                        return
                    qln = tmpp.tile([128, 512], F32, tag="lntmp")
                    nc.vector.tensor_sub(qln[:], t1[:], b_sb[:])
                    # Deinterleaved RoPE: halves e=[0:64], o=[64:128].
                    # Each DVE op keeps both inputs at the same base
                    # partition (walrus constraint); outputs may shift.
                    cs = cos2_t[:, j * 512:(j + 1) * 512]
                    sn = sin2_t[:, j * 512:(j + 1) * 512]
                    m1e = tmpp.tile([64, 512], F32, tag="lnh", bufs=6)
                    nc.vector.tensor_mul(m1e[:], qln[0:64, :], cs[0:64, :])
                    m1o = tmpp.tile([64, 512], F32, tag="lnh", bufs=6)
                    nc.vector.tensor_mul(m1o[:], qln[64:128, :], cs[64:128, :])
                    m2e = tmpp.tile([64, 512], F32, tag="lnh", bufs=6)
                    nc.vector.tensor_mul(m2e[:], qln[0:64, :], sn[0:64, :])
                    m2o = tmpp.tile([64, 512], F32, tag="lnh", bufs=6)
                    nc.vector.tensor_mul(m2o[:], qln[64:128, :], sn[64:128, :])
                    nc.vector.tensor_sub(dst[0:64, dst_col:dst_col + 512],
                                         m1e[:], m2o[:])
                    nc.vector.tensor_add(dst[64:128, dst_col:dst_col + 512],
                                         m2e[:], m1o[:])

                for b in range(B):
                    for hl in range(HPC):
                        hs = hl * 128
                        q_f = bigp.tile([128, S], F32R, tag="qf")
                        k_f = bigp.tile([128, S], F32R, tag="kf")
                        yk_f = ykfp.tile([128, LY], F32R, tag="ykf")
                        # k first: the first QK needs ALL of k_f but only
                        # q chunk 0, so finishing k early starts PE sooner
                        NHALF = 2 if S >= 1024 else 1
                        for src_dr_, dst_f, base_ in ((k_raw_dr, k_f, 32),
                                                      (q_raw_dr, q_f, 0)):
                          for half in range(NHALF):
                            HS2 = S // NHALF
                            c0 = b * S + half * HS2
                            raw_h = lnp.tile([128, HS2], F32, tag="lnraw",
                                             bufs=3, name="rawh")
                            nc.sync.dma_start(
                                raw_h[:], src_dr_[hs:hs + 128, c0:c0 + HS2])
                            for jj in range(HS2 // 512):
                                j = half * (HS2 // 512) + jj
                                ln_chunk(dst_f, j * 512, raw_h, base_, hl,
                                         b * S, j, True, jj)
                        # yk LN (LY <= 512: single chunk)
                        col = b * LY
                        raw = lnp.tile([128, LY], F32, tag="lnrawy")
                        nc.sync.dma_start(raw[:],
                                          yk_raw_dr[hs:hs + 128, col:col + LY])
                        a_ps = coefp.tile([128, LY], F32, tag="s")
                        nc.tensor.matmul(a_ps[:], gam_t[64:65, hs:hs + 128],
                                         rs_t[64:65, col:col + LY],
                                         start=True, stop=True)
                        b_ps = coefp.tile([128, LY], F32, tag="s")
                        nc.tensor.matmul(b_ps[:], gam_t[64:65, hs:hs + 128],
                                         mrs_t[64:65, col:col + LY],
                                         start=True, stop=True)
                        a_sb = tmpp.tile([128, LY], F32, tag="coefsby", bufs=2)
                        nc.scalar.copy(a_sb[:], a_ps[:])
                        b_sb = tmpp.tile([128, LY], F32, tag="coefsby", bufs=2)
                        nc.scalar.activation(b_sb[:], b_ps[:], AF.Identity,
                                             bias=nbcol_t[:, 2 * HPC + hl:
                                                          2 * HPC + hl + 1],
                                             scale=1.0)
                        t1 = tmpp.tile([128, LY], F32, tag="lntmpy")
                        nc.vector.tensor_mul(t1[:], raw[:], a_sb[:])
                        nc.vector.tensor_sub(yk_f[:], t1[:], b_sb[:])

                        v_sb = vp.tile([128, NT * 128], F32R, tag="v")
                        nc.scalar.dma_start(
                            v_sb[:].rearrange("p (t d) -> p t d", t=NT),
                            v_dr[b * S:(b + 1) * S, hs:hs + 128]
                            .rearrange("(t p) d -> p t d", p=128)
                            .bitcast(F32R))
                        vt = [v_sb[:, t * 128:(t + 1) * 128]
                              for t in range(NT)]
                        yv_sb = yvp.tile([128, NTY * 128], F32R, tag="yv")
                        nc.scalar.dma_start(
                            yv_sb[:].rearrange("p (t d) -> p t d", t=NTY),
                            yv_dr[b * LY:(b + 1) * LY, hs:hs + 128]
                            .rearrange("(t p) d -> p t d", p=128)
                            .bitcast(F32R))
                        yvt = [yv_sb[:, t * 128:(t + 1) * 128]
                               for t in range(NTY)]

                        for j in range(NJ):
                            qsl = q_f[:, j * 512:(j + 1) * 512]
                            O_ps = OpsP.tile([128, 512], F32, tag="O")
                            Os_ps = sumP.tile([1, 512], F32, tag="sum")
                            for t in range(NT):
                                s_ps = sp_.tile([128, 512], F32, tag="s")
                                nc.tensor.matmul(
                                    s_ps[:], k_f[:, t * 128:(t + 1) * 128],
                                    qsl, start=True, stop=True)
                                p_t = ptp.tile([128, 512], F32R, tag="p")
                                nc.scalar.activation(p_t[:], s_ps[:], AF.Exp)
                                nc.tensor.matmul(O_ps[:], vt[t], p_t[:],
                                                 start=(t == 0),
                                                 stop=(t == NT - 1))
                                nc.tensor.matmul(Os_ps[:], ones_row[:], p_t[:],
                                                 start=(t == 0),
                                                 stop=(t == NT - 1))
                            O2_ps = O2psP.tile([128, 512], F32, tag="O2")
                            O2s_ps = sum2P.tile([1, 512], F32, tag="sum2")
                            for t in range(NTY):
                                s_ps = sp_.tile([128, 512], F32, tag="s")
                                nc.tensor.matmul(
                                    s_ps[:], yk_f[:, t * 128:(t + 1) * 128],
                                    qsl, start=True, stop=True)
                                p_t = ptp.tile([128, 512], F32R, tag="p")
                                nc.scalar.activation(p_t[:], s_ps[:], AF.Exp)
                                nc.tensor.matmul(O2_ps[:], yvt[t], p_t[:],
                                                 start=(t == 0),
                                                 stop=(t == NTY - 1))
                                nc.tensor.matmul(O2s_ps[:], ones_row[:],
                                                 p_t[:], start=(t == 0),
                                                 stop=(t == NTY - 1))
                            rc1 = rcp.tile([1, 512], F32R, tag="rc")
                            with nc.allow_low_precision(
                                    reason="f32r holds full f32 bits"):
                                nc.vector.reciprocal(rc1[:], Os_ps[:])
                            rc2 = rcp.tile([1, 512], F32R, tag="rc")
                            with nc.allow_low_precision(
                                    reason="f32r holds full f32 bits"):
                                nc.vector.reciprocal(rc2[:], O2s_ps[:])
                            r1_ps = sp_.tile([128, 512], F32, tag="s")
                            nc.tensor.matmul(r1_ps[:], ones_col[:], rc1[:],
                                             start=True, stop=True)
                            r2_ps = sp_.tile([128, 512], F32, tag="s")
                            nc.tensor.matmul(r2_ps[:], g_rows[hl][:], rc2[:],
                                             start=True, stop=True)
                            r1_sb = tmpp.tile([128, 512], F32, tag="lntmp")
                            nc.vector.tensor_copy(r1_sb[:], r1_ps[:])
                            r2_sb = tmpp.tile([128, 512], F32, tag="lntmp")
                            nc.vector.tensor_copy(r2_sb[:], r2_ps[:])
                            o1 = obp.tile([128, 512], F32, tag="ob")
                            nc.vector.tensor_mul(o1[:], O_ps[:], r1_sb[:])
                            o2 = obp.tile([128, 512], F32, tag="ob")
                            nc.vector.tensor_mul(o2[:], O2_ps[:], r2_sb[:])
                            of = obp.tile([128, 512], F32, tag="ob")
                            nc.vector.tensor_add(of[:], o1[:], o2[:])
                            nc.sync.dma_start(
                                o_dr[hs:hs + 128,
                                     b * S + j * 512:b * S + (j + 1) * 512],
                                of[:])

            def _rs(r0, nrows, o0):
                if bench_mode:
                    nc.sync.dma_start(
                        rsout_d[o0:o0 + nrows // NCORES, :],
                        part_dr[r0:r0 + nrows // NCORES, :])
                else:
                    nc.gpsimd.collective_compute(
                        "ReduceScatter", ALU.add,
                        replica_groups=[list(range(NCORES))],
                        ins=[part_dr[r0:r0 + nrows, :].opt()],
                        outs=[rsout_d[o0:o0 + nrows // NCORES, :].opt()])

            # =================== PHASE 3: output projection =================
            with ExitStack() as _s4:
                wop = _s4.enter_context(tc.tile_pool(name="wo", bufs=1))
                otp = _s4.enter_context(tc.tile_pool(name="ot", bufs=6))
                outp = _s4.enter_context(tc.tile_pool(name="outs", bufs=3))
                ops3 = _s4.enter_context(tc.tile_pool(name="ops3", bufs=2, space="PSUM"))
                wo16 = wop.tile([128, 2 * D], F16, tag="wo16")
                with tc.tile_pool(name="wodec", bufs=2) as wodp:
                    for cb in range(2):
                        dec12(wodp, wo16[:, cb * D:(cb + 1) * D],
                              bview8("wo_hi", 128, cb, 2),
                              bview8("wo_lo", 128, cb, 2), 128, D)
                wo_sb = wop.tile([128, 2 * D], F32R, tag="wo")
                nc.vector.tensor_copy(wo_sb[:], wo16[:])
                for rg in range(R // 512):
                  o_ts = []
                  for cb in range(2):
                    o_t = otp.tile([128, 512], F32R, tag="ot")
                    nc.sync.dma_start(
                        o_t[:],
                        o_dr[cb * 128:(cb + 1) * 128,
                             rg * 512:(rg + 1) * 512].bitcast(F32R))
                    o_ts.append(o_t)
                  for rt4 in range(4):
                    rt = rg * 4 + rt4
                    ob_ = outp.tile([128, D], F16, tag="outsb")
                    for oc in range(D // 512):
                        ps = ops3.tile([128, 512], F32, tag="out")
                        for cb in range(2):
                            nc.tensor.matmul(
                                ps[:],
                                o_ts[cb][:, rt4 * 128:(rt4 + 1) * 128],
                                wo_sb[:, cb * D + oc * 512:
                                      cb * D + (oc + 1) * 512],
                                start=(cb == 0), stop=(cb == 1))
                        if oc % 2 == 0:
                            nc.scalar.copy(ob_[:, oc * 512:(oc + 1) * 512],
                                           ps[:])
                        else:
                            nc.vector.tensor_copy(
                                ob_[:, oc * 512:(oc + 1) * 512], ps[:])
                    nc.scalar.dma_start(part_dr[rt * 128:(rt + 1) * 128, :],
                                        ob_[:])
                  # half-way: reduce-scatter the finished first half so the
                  # collective overlaps the second half's matmuls. Core c
                  # gets global rows [c*RSH/2, (c+1)*RSH/2) of each half
                  # (host reorders; see unshard_out).
                  if rg == R // 1024 - 1:
                      _rs(0, R // 2, 0)
                _rs(R // 2, R // 2, RSH // 2)

            # pack the f16 shard to 12-bit planes (round-half-up)
            with tc.tile_pool(name="opack", bufs=3) as opk:
                for rb in range(RSH // 128):
                    f16t = opk.tile([128, D], F16, tag="pk_in")
                    nc.sync.dma_start(
                        f16t[:], rsout_d[rb * 128:(rb + 1) * 128, :])
                    rnd = opk.tile([128, D], U16, tag="pk_rnd")
                    nc.vector.tensor_scalar(
                        out=rnd[:], in0=f16t[:].bitcast(U16), scalar1=8,
                        scalar2=None, op0=ALU.add)
                    hi16 = opk.tile([128, D], U16, tag="pk_hi16")
                    nc.vector.tensor_scalar(
                        out=hi16[:], in0=rnd[:], scalar1=8, scalar2=None,
                        op0=ALU.logical_shift_right)
                    hi8 = opk.tile([128, D], U8, tag="pk_hi8")
                    nc.vector.tensor_copy(hi8[:], hi16[:])
                    nc.sync.dma_start(
                        out_d[0:1, rb * 128 * D // 2:(rb + 1) * 128 * D // 2]
                        .bitcast(U8).rearrange("p (a b) -> (p a) b", a=128),
                        hi8[:])
                    rv = rnd[:].rearrange("p (n two) -> p n two", two=2)
                    t0 = opk.tile([128, D // 2], U16, tag="pk_t0")
                    nc.vector.tensor_scalar(
                        out=t0[:].rearrange("p (n o) -> p n o", o=1),
                        in0=rv[:, :, 0:1], scalar1=4, scalar2=0xF,
                        op0=ALU.logical_shift_right, op1=ALU.bitwise_and)
                    t1 = opk.tile([128, D // 2], U16, tag="pk_t1")
                    nc.vector.tensor_scalar(
                        out=t1[:].rearrange("p (n o) -> p n o", o=1),
                        in0=rv[:, :, 1:2], scalar1=0xF0, scalar2=None,
                        op0=ALU.bitwise_and)
                    lo16 = opk.tile([128, D // 2], U16, tag="pk_lo16")
                    nc.vector.tensor_tensor(out=lo16[:], in0=t0[:],
                                            in1=t1[:], op=ALU.bitwise_or)
                    lo8 = opk.tile([128, D // 2], U8, tag="pk_lo8")
                    nc.vector.tensor_copy(lo8[:], lo16[:])
                    nc.sync.dma_start(
                        out_d[0:1, NOUT_HI + rb * 128 * D // 4:
                              NOUT_HI + (rb + 1) * 128 * D // 4]
                        .bitcast(U8).rearrange("p (a b) -> (p a) b", a=128),
                        lo8[:])

    nc.compile()
    return nc


def _perm_for_core(c):
    idx = []
    for h in (HPC * c + i for i in range(HPC)):
        base = h * HD_F
        idx.extend(base + np.arange(0, HD_F, 2))
        idx.extend(base + np.arange(1, HD_F, 2))
    return np.array(idx)


def make_in_maps(cfg, inputs):
    B, S, D, LY, DY = cfg["B"], cfg["S"], cfg["D"], cfg["LY"], cfg["DY"]
    R, RY = B * S, B * LY
    DCH, DYCH = D // NCORES, DY // NCORES
    f32 = np.float32
    f16 = np.float16
    x = np.asarray(inputs["x"], f32)
    y = np.asarray(inputs["y"], f32)
    fc = np.asarray(inputs["freqs_cis"], f32)      # [S, 64, 2]
    wq = np.asarray(inputs["wq"], f32)
    wk = np.asarray(inputs["wk"], f32)
    wv = np.asarray(inputs["wv"], f32)
    wo = np.asarray(inputs["wo"], f32)
    wky = np.asarray(inputs["wky"], f32)
    wvy = np.asarray(inputs["wvy"], f32)
    gate = np.asarray(inputs["gate"], f32)
    qn_w = np.asarray(inputs["qn_w"], f32)
    qn_b = np.asarray(inputs["qn_b"], f32)
    kn_w = np.asarray(inputs["kn_w"], f32)
    kn_b = np.asarray(inputs["kn_b"], f32)
    kyn_w = np.asarray(inputs["kyn_w"], f32)
    kyn_b = np.asarray(inputs["kyn_b"], f32)

    xT = np.ascontiguousarray(x.reshape(R, D).T.astype(f16))
    yT = np.ascontiguousarray(y.reshape(RY, DY).T.astype(f16))
    cosv = fc[:, :, 0].T                           # [64, S]
    sinv = fc[:, :, 1].T
    cs_full = np.ascontiguousarray(
        np.concatenate([cosv, sinv], axis=0).astype(f16))   # [128, S]
    CSCH = 128 // NCORES
    scale = 1.0 / math.sqrt(HD_F)
    lay, ntot = _blob_layout(cfg)

    in_maps = []
    for c in range(NCORES):
        perm = _perm_for_core(c)
        nat = np.arange(c * C, (c + 1) * C)
        gam = np.zeros((65, C), f32)
        gam[0] = qn_w[perm] * scale
        gam[32] = kn_w[perm]
        gam[64] = kyn_w[perm]
        nbcol = np.zeros((128, 3 * HPC), f32)
        for i in range(HPC):
            sl = slice(i * 128, (i + 1) * 128)
            nbcol[:, 0 * HPC + i] = -qn_b[perm][sl] * scale
            nbcol[:, 1 * HPC + i] = -kn_b[perm][sl]
            nbcol[:, 2 * HPC + i] = -kyn_b[perm][sl]
        gate_65 = np.zeros((65, 1), f32)
        for i in range(HPC):
            gate_65[32 * i, 0] = gate[HPC * c + i]
        packed = dict(
            xg=xT[c * DCH:(c + 1) * DCH, :],
            yg=yT[c * DYCH:(c + 1) * DYCH, :],
            wq=wq[:, perm], wk=wk[:, perm], wv=wv[:, nat],
            wky=wky[:, perm], wvy=wvy[:, nat], wo=wo[nat, :],
        )
        regions = dict(
            csg=cs_full[c * CSCH:(c + 1) * CSCH, :],
            gam=gam, nbcol=nbcol, gate=gate_65,
        )
        for name, arr in packed.items():
            hi, lo = _pack12(arr)
            regions[name + "_hi"] = hi
            regions[name + "_lo"] = lo
        blob = np.zeros((1, ntot), f16)
        for name, arr in regions.items():
            off, n = lay[name]
            flat = np.ascontiguousarray(arr)
            if flat.dtype == np.float32:
                flat = flat.view(f16)
            elif flat.dtype == np.uint8:
                if flat.size % 2:
                    flat = np.concatenate([flat.ravel(), flat.ravel()[:1] * 0])
                flat = flat.view(f16)
            blob[0, off:off + n] = flat.ravel()
        in_maps.append(dict(blob=blob))
    return in_maps


def unshard_out(parts, cfg):
    """parts[c] = core c's 12-bit-packed [1, NOUT] f16 buffer (hi-byte
    plane then low-nibble-pair plane) holding its [RSH, D] shard; rows of
    each half-RS chunk interleave across cores (half1 chunk c = global
    rows [c*RSH/2, ...), half2 chunk c = global rows [R/2 + c*RSH/2, ...))."""
    R, D = cfg["B"] * cfg["S"], cfg["D"]
    RSH = R // NCORES
    H2 = RSH // 2
    out = np.empty((R, D), np.float32)
    for c, p in enumerate(parts):
        by = np.ascontiguousarray(np.asarray(p)).view(np.uint8).ravel()
        hi = by[:RSH * D].astype(np.uint16)
        lo = by[RSH * D:RSH * D + RSH * D // 2].astype(np.uint16)
        bits = hi << 8
        bits[0::2] |= (lo & 0xF) << 4
        bits[1::2] |= lo & 0xF0
        sh = bits.view(np.float16).reshape(RSH, D).astype(np.float32)
        out[c * H2:(c + 1) * H2] = sh[:H2]
        out[R // 2 + c * H2:R // 2 + (c + 1) * H2] = sh[H2:]
    return out.reshape(cfg["B"], cfg["S"], cfg["D"])


def kernel(**inputs):
    from concourse.bass_utils import run_bass_kernel_spmd
    cfg = _cfg_full()
    key = tuple(sorted(cfg.items()))
    if key not in _BUILD_CACHE:
        _BUILD_CACHE[key] = build(cfg)
    nc = _BUILD_CACHE[key]
    in_maps = make_in_maps(cfg, inputs)
    try:
        res = run_bass_kernel_spmd(nc, in_maps, list(range(NCORES)),
                                   trace=TRACE)
    except ModuleNotFoundError:
        res = run_bass_kernel_spmd(nc, in_maps, list(range(NCORES)))
    out = unshard_out([r["out_sl"] for r in res.results], cfg)
    kernel._last_result = res
    return out


kernel._last_result = None
